# revision 23
# baseline (speedup 1.0000x reference)
"""Trainium2 Bass kernel for nn_LinearReg_55508157333593.

Computes: loss = (c_omega * 0.001 / N) * sum over all rows/groups of
L2 norms of 25-element groups of weight [100000, 800] f32.

Strategy (rates measured on HW):
- Memory-bound problem; tolerance 2e-2 admits quantized upload. Each
  chunk is stored k-major (chunk = [slice0 | ... | slice24], slice k
  holding element k of the chunk's Gc groups) so the group reduction
  becomes contiguous adds.
- Mixed-dtype upload: ACT's share of each chunk goes up as fp8 e4m3
  (ACT Square runs 0.86 ns/elem at any dtype), DVE's share goes up as
  bf16 so its tensor_tensor square hits the 2x DVE mode (0.53 ns/elem
  vs 1.05 for fp8 input). DMA has ~25 us of slack, so the extra bf16
  bytes are free. GpSimd was tried and removed: its concurrent
  traffic degraded DVE adds 2-6x.
- Group reduction on DVE: fold-add tree of CONTIGUOUS OUT-OF-PLACE
  bf16 adds (2x mode; TensorReduce is always 1x, in-place adds ~2x
  slower). 6 instructions, 24G adds per chunk; last level sums into
  f32 gs_all to limit bf16 truncation bias.
- ACT-only tail: the last chunks are squared entirely by ACT (which
  idles waiting for the sqrt segment there) so DVE finishes folds
  sooner.
- Endgame: ACT sqrt over gs_all segments (in-place, f32) with fused
  per-partition accumulation into pr [128, n_segs]; pr is DMA'd out
  directly and the host does the final partition/core sum in f64 and
  applies (0.001 * c_omega / N).
"""

import sys

import numpy as np

if "/opt/trn_rl_repo" not in sys.path:
    sys.path.insert(0, "/opt/trn_rl_repo")

N_CORES = 8
P = 128
GROUP = 25
C_OMEGA = 0.001
N_ROWS = 100000
ROW = 800
F_PER_PART = (N_ROWS * ROW) // (N_CORES * P)   # 78125 elems/partition/core

# chunk schedule (elems per partition, multiples of 25, sum 78125).
# small first chunk -> compute starts early; descending tail -> short
# serial chain after the last DMA byte.
SCHEDULE = [1250, 5000, 9375, 12000, 12000, 12000, 12000, 9375, 3750,
            1250, 125]
N_SLOTS = 4              # square-buffer / bf16-staging ring depth
ACT_FRAC = 0.82          # ACT's share of each chunk's squares
ACT_ONLY_TAIL = 3        # last chunks squared entirely by ACT
# sqrt segments: (needs folds of chunks < n, emitted after square n-1)
SEG_PLAN = [(9, 11), (11, 11)]

_compiled = None
LAST_RESULTS = None


def _chunk_layout(schedule):
    n = len(schedule)
    offs = np.cumsum([0] + list(schedule))
    gcs = [c // GROUP for c in schedule]
    goffs = np.cumsum([0] + gcs)
    return n, offs, gcs, goffs


def _splits(schedule, act_frac):
    """Per-chunk ACT/DVE column split + fp8/bf16 stream offsets."""
    n = len(schedule)
    a_split = [min(c, max(0, int(round(c * act_frac / 4)) * 4))
               for c in schedule]
    for j in range(max(0, n - ACT_ONLY_TAIL), n):
        a_split[j] = schedule[j]
    st = [c - a for c, a in zip(schedule, a_split)]
    off8 = np.cumsum([0] + a_split)
    offb = np.cumsum([0] + st)
    return a_split, st, off8, offb


def build(f_per_part=F_PER_PART, schedule=None, act_frac=ACT_FRAC,
          seg_plan=None):
    from concourse import bacc, mybir

    if schedule is None:
        schedule = SCHEDULE
        seg_plan = SEG_PLAN
    n, offs, gcs, goffs = _chunk_layout(schedule)
    total_g = int(goffs[n])
    assert sum(schedule) == f_per_part
    assert all(c % GROUP == 0 for c in schedule)
    if seg_plan is None:
        seg_plan = [(n, n)]
    assert seg_plan[-1][0] == n
    max_c = max(schedule)
    a_split, st, off8, offb = _splits(schedule, act_frac)
    S8 = int(off8[n])
    SB = int(offb[n])
    max_st = max(st)

    f32 = mybir.dt.float32
    bf16 = mybir.dt.bfloat16
    fp8 = mybir.dt.float8e4
    Act = mybir.ActivationFunctionType
    Alu = mybir.AluOpType

    nc = bacc.Bacc("TRN2", target_bir_lowering=False, debug=False,
                   num_devices=N_CORES)
    x8 = nc.dram_tensor("x8", [P, S8], fp8, kind="ExternalInput").ap()
    xb = (nc.dram_tensor("xb", [P, SB], bf16, kind="ExternalInput").ap()
          if SB else None)
    n_segs = len(seg_plan)
    out = nc.dram_tensor("out", [P, n_segs], f32, kind="ExternalOutput").ap()

    max_g = max_c // GROUP
    fa = nc.alloc_sbuf_tensor("fa", [P, 12 * max_g], bf16).ap()
    fb = nc.alloc_sbuf_tensor("fb", [P, 6 * max_g], bf16).ap()
    gs_all = nc.alloc_sbuf_tensor("gs_all", [P, total_g], f32).ap()
    xs8 = nc.alloc_sbuf_tensor("xs8", [P, S8], fp8).ap()
    stg = ([nc.alloc_sbuf_tensor(f"stg{b}", [P, max_st], bf16).ap()
            for b in range(N_SLOTS)] if max_st else None)
    sq = [nc.alloc_sbuf_tensor(f"sq{b}", [P, max_c], bf16).ap()
          for b in range(N_SLOTS)]
    pr = nc.alloc_sbuf_tensor("pr", [P, n_segs], f32).ap()
    dm = nc.alloc_sbuf_tensor("dm_scratch", [1, 1], f32).ap()
    ones = nc.const_aps.aps[(f32, 1.0)]

    dma_sems = [nc.alloc_semaphore(f"dma_sem{i}") for i in range(n)]
    act_sem = nc.alloc_semaphore("act_sem")
    mult_sem = nc.alloc_semaphore("mult_sem")
    fold_sem = nc.alloc_semaphore("fold_sem")
    sqrt_sem = nc.alloc_semaphore("sqrt_sem")
    out_sem = nc.alloc_semaphore("out_sem")

    # dma_sems[i] target: 16 per dma_start issued for chunk i
    dma_need = [16 + (16 if st[i] else 0) for i in range(n)]

    # ---- SP: input DMAs (fp8 + bf16 streams), then result out ----
    sp = nc.sync
    for i in range(n):
        if a_split[i]:
            sp.dma_start(xs8[:, off8[i]:off8[i + 1]],
                         x8[:, off8[i]:off8[i + 1]]).then_inc(dma_sems[i], 16)
        if st[i]:
            if i >= N_SLOTS:
                # stage slot reuse: DVE must have consumed slot i-N_SLOTS
                sp.wait_ge(mult_sem, i - N_SLOTS + 1)
            sp.dma_start(stg[i % N_SLOTS][:, :st[i]],
                         xb[:, offb[i]:offb[i + 1]]).then_inc(dma_sems[i], 16)
    sp.wait_ge(sqrt_sem, n_segs)
    sp.dma_start(out, pr).then_inc(out_sem, 16)
    sp.wait_ge(out_sem, 16)

    # ---- ACT: table load, squares (fp8 share), sqrt segs ----
    act = nc.scalar
    act.activation(dm, ones[0:1, :], Act.Sqrt)   # table prefetch

    seg_by_after = {}
    prev = 0
    for s, (need, after) in enumerate(seg_plan):
        glo, ghi = int(goffs[prev]), int(goffs[need])
        seg_by_after.setdefault(after, []).append((s, need, glo, ghi))
        prev = need

    def emit_segs(after_idx):
        for s, need, glo, ghi in seg_by_after.get(after_idx, []):
            act.wait_ge(fold_sem, need)
            act.activation(gs_all[:, glo:ghi], gs_all[:, glo:ghi], Act.Sqrt,
                           accum_out=pr[:, s:s + 1]).then_inc(sqrt_sem, 1)

    for i in range(n):
        if i >= N_SLOTS:
            act.wait_ge(fold_sem, i - N_SLOTS + 1)
        act.wait_ge(dma_sems[i], dma_need[i])
        a = a_split[i]
        if a > 0:
            act.activation(sq[i % N_SLOTS][:, :a],
                           xs8[:, off8[i]:off8[i + 1]],
                           Act.Square).then_inc(act_sem, 1)
        else:
            act.activation(dm, ones[0:1, :], Act.Sqrt).then_inc(act_sem, 1)
        emit_segs(i + 1)
    emit_segs(n + 1)

    # ---- DVE: bf16 squares (2x mult) + fold tree per chunk ----
    # one-chunk mult lookahead so DVE never idles waiting for ACT
    dve = nc.vector

    def emit_mult(j):
        if st[j]:
            dve.wait_ge(dma_sems[j], dma_need[j])
            s_in = stg[j % N_SLOTS][:, :st[j]]
            dve.tensor_tensor(sq[j % N_SLOTS][:, a_split[j]:schedule[j]],
                              s_in, s_in, op=Alu.mult).then_inc(mult_sem, 1)
        else:
            dve.tensor_copy(dm, dm).then_inc(mult_sem, 1)

    emit_mult(0)
    for i in range(n):
        g = gcs[i]
        s = sq[i % N_SLOTS]
        if i + 1 < n:
            emit_mult(i + 1)
        dve.wait_ge(act_sem, i + 1)
        # k-major chunk: 25 slices of g elems; contiguous out-of-place
        # ping-pong folds (in-place adds measured ~2x slower on HW)
        dve.tensor_tensor(fa[:, 0:12 * g], s[:, 0:12 * g],
                          s[:, 12 * g:24 * g], op=Alu.add)
        dve.tensor_tensor(fb[:, 0:6 * g], fa[:, 0:6 * g],
                          fa[:, 6 * g:12 * g], op=Alu.add)
        dve.tensor_tensor(fa[:, 0:3 * g], fb[:, 0:3 * g],
                          fb[:, 3 * g:6 * g], op=Alu.add)
        dve.tensor_tensor(fb[:, 0:g], fa[:, 0:g],
                          fa[:, g:2 * g], op=Alu.add)
        dve.tensor_tensor(fb[:, g:2 * g], fb[:, 0:g],
                          fa[:, 2 * g:3 * g], op=Alu.add)
        dve.tensor_tensor(gs_all[:, goffs[i]:goffs[i + 1]],
                          fb[:, g:2 * g],
                          s[:, 24 * g:25 * g], op=Alu.add).then_inc(fold_sem, 1)

    nc.compile()
    return nc


def _prep_streams(wf, schedule, act_frac):
    """wf: float32 [..., f_per_part]. Returns (x8 bytes, xb bf16) streams
    with each chunk k-major and split into ACT(fp8)/DVE(bf16) parts."""
    import ml_dtypes

    n, offs, gcs, goffs = _chunk_layout(schedule)
    a_split, st, off8, offb = _splits(schedule, act_frac)
    lead = wf.shape[:-1]
    x8 = np.empty(lead + (int(off8[n]),), np.uint8)
    xb = np.empty(lead + (int(offb[n]),), ml_dtypes.bfloat16)
    for i in range(n):
        blk = wf[..., offs[i]:offs[i + 1]]
        km = (blk.reshape(lead + (gcs[i], GROUP))
              .swapaxes(-1, -2).reshape(lead + (schedule[i],)))
        a = a_split[i]
        x8[..., off8[i]:off8[i + 1]] = (
            km[..., :a].astype(ml_dtypes.float8_e4m3).view(np.uint8))
        if st[i]:
            xb[..., offb[i]:offb[i + 1]] = km[..., a:].astype(
                ml_dtypes.bfloat16)
    return x8.view(ml_dtypes.float8_e4m3), xb


def kernel(weight, c_omega):
    global _compiled, LAST_RESULTS
    from concourse.bass_utils import run_bass_kernel_spmd

    if _compiled is None:
        _compiled = build()
    nc = _compiled

    w = np.asarray(weight)
    if w.dtype != np.float32:
        w = w.astype(np.float32)
    wf = np.ascontiguousarray(w).reshape(N_CORES, P, F_PER_PART)
    x8, xb = _prep_streams(wf, SCHEDULE, ACT_FRAC)
    in_maps = [{"x8": x8[c], "xb": xb[c]} for c in range(N_CORES)]
    LAST_RESULTS = run_bass_kernel_spmd(nc, in_maps,
                                        core_ids=list(range(N_CORES)))
    total = 0.0
    for r in LAST_RESULTS.results:
        total += float(np.asarray(r["out"]).astype(np.float64).sum())
    loss = total / N_ROWS * (C_OMEGA * float(c_omega))
    return np.float32(loss)


def selftest_sim(f_per_part=625, schedule=(125, 150, 100, 125, 75, 25, 25),
                 seg_plan=((6, 7), (7, 7)), seed=0):
    """CoreSim numeric check on a scaled-down instance."""
    from concourse.bass_interp import CoreSim

    nc = build(f_per_part=f_per_part, schedule=list(schedule),
               seg_plan=[tuple(x) for x in seg_plan])
    nc.detect_race_conditions = False
    rng = np.random.default_rng(seed)
    wf = rng.standard_normal((P, f_per_part)).astype(np.float32)
    x8, xb = _prep_streams(wf, list(schedule), ACT_FRAC)
    sim = CoreSim(nc)
    sim.tensor("x8")[:] = x8
    if xb.shape[-1]:
        sim.tensor("xb")[:] = xb
    sim.simulate()
    got = float(np.array(sim.tensor("out")).astype(np.float64).sum())
    g = wf.astype(np.float64).reshape(P, f_per_part // GROUP, GROUP)
    want = float(np.sqrt((g ** 2).sum(-1)).sum())
    return abs(got - want) / abs(want)


# revision 25
# speedup vs baseline: 1.0091x; 1.0091x over previous
"""Trainium2 Bass kernel for nn_LinearReg_55508157333593.

Computes: loss = (c_omega * 0.001 / N) * sum over all rows/groups of
L2 norms of 25-element groups of weight [100000, 800] f32.

Strategy (rates measured on HW):
- Memory-bound problem; tolerance 2e-2 admits quantized upload. Each
  chunk is stored k-major (chunk = [slice0 | ... | slice24], slice k
  holding element k of the chunk's Gc groups) so the group reduction
  becomes contiguous adds.
- Mixed-dtype upload: ACT's share of each chunk goes up as fp8 e4m3
  (ACT Square runs 0.86 ns/elem at any dtype), DVE's share goes up as
  bf16 so its tensor_tensor square hits the 2x DVE mode (0.53 ns/elem
  vs 1.05 for fp8 input). DMA has ~25 us of slack, so the extra bf16
  bytes are free. GpSimd was tried and removed: its concurrent
  traffic degraded DVE adds 2-6x.
- Group reduction on DVE: fold-add tree of CONTIGUOUS OUT-OF-PLACE
  bf16 adds (2x mode; TensorReduce is always 1x, in-place adds ~2x
  slower). 6 instructions, 24G adds per chunk; last level sums into
  f32 gs_all to limit bf16 truncation bias.
- ACT-only tail: the last chunks are squared entirely by ACT (which
  idles waiting for the sqrt segment there) so DVE finishes folds
  sooner.
- Endgame: ACT sqrt over gs_all segments (in-place, f32) with fused
  per-partition accumulation into pr [128, n_segs]; pr is DMA'd out
  directly and the host does the final partition/core sum in f64 and
  applies (0.001 * c_omega / N).
"""

import sys

import numpy as np

if "/opt/trn_rl_repo" not in sys.path:
    sys.path.insert(0, "/opt/trn_rl_repo")

N_CORES = 8
P = 128
GROUP = 25
C_OMEGA = 0.001
N_ROWS = 100000
ROW = 800
F_PER_PART = (N_ROWS * ROW) // (N_CORES * P)   # 78125 elems/partition/core

# chunk schedule (elems per partition, multiples of 25, sum 78125).
# small first chunk -> compute starts early; descending tail -> short
# serial chain after the last DMA byte.
SCHEDULE = [1250, 5000, 9375, 12000, 12000, 12000, 12000, 9375, 3750,
            1250, 125]
N_SLOTS = 4              # square-buffer / bf16-staging ring depth
ACT_FRAC = 0.80          # ACT's share of each chunk's squares
ACT_ONLY_TAIL = 3        # last chunks squared entirely by ACT
ACT_ONLY_HEAD = 2        # first chunks too (single DMA -> earliest start)
# sqrt segments: (needs folds of chunks < n, emitted after square n-1)
SEG_PLAN = [(9, 11), (11, 11)]

_compiled = None
LAST_RESULTS = None


def _chunk_layout(schedule):
    n = len(schedule)
    offs = np.cumsum([0] + list(schedule))
    gcs = [c // GROUP for c in schedule]
    goffs = np.cumsum([0] + gcs)
    return n, offs, gcs, goffs


def _splits(schedule, act_frac):
    """Per-chunk ACT/DVE column split + fp8/bf16 stream offsets."""
    n = len(schedule)
    a_split = [min(c, max(0, int(round(c * act_frac / 4)) * 4))
               for c in schedule]
    for j in range(max(0, n - ACT_ONLY_TAIL), n):
        a_split[j] = schedule[j]
    for j in range(min(ACT_ONLY_HEAD, n)):
        a_split[j] = schedule[j]
    st = [c - a for c, a in zip(schedule, a_split)]
    off8 = np.cumsum([0] + a_split)
    offb = np.cumsum([0] + st)
    return a_split, st, off8, offb


def build(f_per_part=F_PER_PART, schedule=None, act_frac=ACT_FRAC,
          seg_plan=None):
    from concourse import bacc, mybir

    if schedule is None:
        schedule = SCHEDULE
        seg_plan = SEG_PLAN
    n, offs, gcs, goffs = _chunk_layout(schedule)
    total_g = int(goffs[n])
    assert sum(schedule) == f_per_part
    assert all(c % GROUP == 0 for c in schedule)
    if seg_plan is None:
        seg_plan = [(n, n)]
    assert seg_plan[-1][0] == n
    max_c = max(schedule)
    a_split, st, off8, offb = _splits(schedule, act_frac)
    S8 = int(off8[n])
    SB = int(offb[n])
    max_st = max(st)

    f32 = mybir.dt.float32
    bf16 = mybir.dt.bfloat16
    fp8 = mybir.dt.float8e4
    Act = mybir.ActivationFunctionType
    Alu = mybir.AluOpType

    nc = bacc.Bacc("TRN2", target_bir_lowering=False, debug=False,
                   num_devices=N_CORES)
    x8 = nc.dram_tensor("x8", [P, S8], fp8, kind="ExternalInput").ap()
    xb = (nc.dram_tensor("xb", [P, SB], bf16, kind="ExternalInput").ap()
          if SB else None)
    n_segs = len(seg_plan)
    out = nc.dram_tensor("out", [P, n_segs], f32, kind="ExternalOutput").ap()

    max_g = max_c // GROUP
    fa = nc.alloc_sbuf_tensor("fa", [P, 12 * max_g], bf16).ap()
    fb = nc.alloc_sbuf_tensor("fb", [P, 6 * max_g], bf16).ap()
    gs_all = nc.alloc_sbuf_tensor("gs_all", [P, total_g], f32).ap()
    xs8 = nc.alloc_sbuf_tensor("xs8", [P, S8], fp8).ap()
    stg = ([nc.alloc_sbuf_tensor(f"stg{b}", [P, max_st], bf16).ap()
            for b in range(N_SLOTS)] if max_st else None)
    sq = [nc.alloc_sbuf_tensor(f"sq{b}", [P, max_c], bf16).ap()
          for b in range(N_SLOTS)]
    pr = nc.alloc_sbuf_tensor("pr", [P, n_segs], f32).ap()
    dm = nc.alloc_sbuf_tensor("dm_scratch", [1, 1], f32).ap()
    ones = nc.const_aps.aps[(f32, 1.0)]

    dma_sems = [nc.alloc_semaphore(f"dma_sem{i}") for i in range(n)]
    act_sem = nc.alloc_semaphore("act_sem")
    mult_sem = nc.alloc_semaphore("mult_sem")
    fold_sem = nc.alloc_semaphore("fold_sem")
    sqrt_sem = nc.alloc_semaphore("sqrt_sem")
    out_sem = nc.alloc_semaphore("out_sem")

    # dma_sems[i] target: 16 per dma_start issued for chunk i
    dma_need = [16 + (16 if st[i] else 0) for i in range(n)]

    # ---- SP: input DMAs (fp8 + bf16 streams), then result out ----
    sp = nc.sync
    for i in range(n):
        if a_split[i]:
            sp.dma_start(xs8[:, off8[i]:off8[i + 1]],
                         x8[:, off8[i]:off8[i + 1]]).then_inc(dma_sems[i], 16)
        if st[i]:
            if i >= N_SLOTS:
                # stage slot reuse: DVE must have consumed slot i-N_SLOTS
                sp.wait_ge(mult_sem, i - N_SLOTS + 1)
            sp.dma_start(stg[i % N_SLOTS][:, :st[i]],
                         xb[:, offb[i]:offb[i + 1]]).then_inc(dma_sems[i], 16)
    sp.wait_ge(sqrt_sem, n_segs)
    sp.dma_start(out, pr).then_inc(out_sem, 16)
    sp.wait_ge(out_sem, 16)

    # ---- ACT: table load, squares (fp8 share), sqrt segs ----
    act = nc.scalar
    act.activation(dm, ones[0:1, :], Act.Sqrt)   # table prefetch

    seg_by_after = {}
    prev = 0
    for s, (need, after) in enumerate(seg_plan):
        glo, ghi = int(goffs[prev]), int(goffs[need])
        seg_by_after.setdefault(after, []).append((s, need, glo, ghi))
        prev = need

    def emit_segs(after_idx):
        for s, need, glo, ghi in seg_by_after.get(after_idx, []):
            act.wait_ge(fold_sem, need)
            act.activation(gs_all[:, glo:ghi], gs_all[:, glo:ghi], Act.Sqrt,
                           accum_out=pr[:, s:s + 1]).then_inc(sqrt_sem, 1)

    for i in range(n):
        if i >= N_SLOTS:
            act.wait_ge(fold_sem, i - N_SLOTS + 1)
        act.wait_ge(dma_sems[i], dma_need[i])
        a = a_split[i]
        if a > 0:
            act.activation(sq[i % N_SLOTS][:, :a],
                           xs8[:, off8[i]:off8[i + 1]],
                           Act.Square).then_inc(act_sem, 1)
        else:
            act.activation(dm, ones[0:1, :], Act.Sqrt).then_inc(act_sem, 1)
        emit_segs(i + 1)
    emit_segs(n + 1)

    # ---- DVE: bf16 squares (2x mult) + fold tree per chunk ----
    # one-chunk mult lookahead so DVE never idles waiting for ACT
    dve = nc.vector

    def emit_mult(j):
        if st[j]:
            dve.wait_ge(dma_sems[j], dma_need[j])
            s_in = stg[j % N_SLOTS][:, :st[j]]
            dve.tensor_tensor(sq[j % N_SLOTS][:, a_split[j]:schedule[j]],
                              s_in, s_in, op=Alu.mult).then_inc(mult_sem, 1)
        else:
            dve.tensor_copy(dm, ones[0:1, :]).then_inc(mult_sem, 1)

    emit_mult(0)
    for i in range(n):
        g = gcs[i]
        s = sq[i % N_SLOTS]
        if i + 1 < n:
            emit_mult(i + 1)
        dve.wait_ge(act_sem, i + 1)
        # k-major chunk: 25 slices of g elems; contiguous out-of-place
        # ping-pong folds (in-place adds measured ~2x slower on HW)
        dve.tensor_tensor(fa[:, 0:12 * g], s[:, 0:12 * g],
                          s[:, 12 * g:24 * g], op=Alu.add)
        dve.tensor_tensor(fb[:, 0:6 * g], fa[:, 0:6 * g],
                          fa[:, 6 * g:12 * g], op=Alu.add)
        dve.tensor_tensor(fa[:, 0:3 * g], fb[:, 0:3 * g],
                          fb[:, 3 * g:6 * g], op=Alu.add)
        dve.tensor_tensor(fb[:, 0:g], fa[:, 0:g],
                          fa[:, g:2 * g], op=Alu.add)
        dve.tensor_tensor(fb[:, g:2 * g], fb[:, 0:g],
                          fa[:, 2 * g:3 * g], op=Alu.add)
        dve.tensor_tensor(gs_all[:, goffs[i]:goffs[i + 1]],
                          fb[:, g:2 * g],
                          s[:, 24 * g:25 * g], op=Alu.add).then_inc(fold_sem, 1)

    nc.compile()
    return nc


def _prep_streams(wf, schedule, act_frac):
    """wf: float32 [..., f_per_part]. Returns (x8 bytes, xb bf16) streams
    with each chunk k-major and split into ACT(fp8)/DVE(bf16) parts."""
    import ml_dtypes

    n, offs, gcs, goffs = _chunk_layout(schedule)
    a_split, st, off8, offb = _splits(schedule, act_frac)
    lead = wf.shape[:-1]
    x8 = np.empty(lead + (int(off8[n]),), np.uint8)
    xb = np.empty(lead + (int(offb[n]),), ml_dtypes.bfloat16)
    for i in range(n):
        blk = wf[..., offs[i]:offs[i + 1]]
        km = (blk.reshape(lead + (gcs[i], GROUP))
              .swapaxes(-1, -2).reshape(lead + (schedule[i],)))
        a = a_split[i]
        x8[..., off8[i]:off8[i + 1]] = (
            km[..., :a].astype(ml_dtypes.float8_e4m3).view(np.uint8))
        if st[i]:
            xb[..., offb[i]:offb[i + 1]] = km[..., a:].astype(
                ml_dtypes.bfloat16)
    return x8.view(ml_dtypes.float8_e4m3), xb


def kernel(weight, c_omega):
    global _compiled, LAST_RESULTS
    from concourse.bass_utils import run_bass_kernel_spmd

    if _compiled is None:
        _compiled = build()
    nc = _compiled

    w = np.asarray(weight)
    if w.dtype != np.float32:
        w = w.astype(np.float32)
    wf = np.ascontiguousarray(w).reshape(N_CORES, P, F_PER_PART)
    x8, xb = _prep_streams(wf, SCHEDULE, ACT_FRAC)
    in_maps = [{"x8": x8[c], "xb": xb[c]} for c in range(N_CORES)]
    LAST_RESULTS = run_bass_kernel_spmd(nc, in_maps,
                                        core_ids=list(range(N_CORES)))
    total = 0.0
    for r in LAST_RESULTS.results:
        total += float(np.asarray(r["out"]).astype(np.float64).sum())
    loss = total / N_ROWS * (C_OMEGA * float(c_omega))
    return np.float32(loss)


def selftest_sim(f_per_part=625, schedule=(125, 150, 100, 125, 75, 25, 25),
                 seg_plan=((6, 7), (7, 7)), seed=0):
    """CoreSim numeric check on a scaled-down instance."""
    from concourse.bass_interp import CoreSim

    nc = build(f_per_part=f_per_part, schedule=list(schedule),
               seg_plan=[tuple(x) for x in seg_plan])
    nc.detect_race_conditions = False
    rng = np.random.default_rng(seed)
    wf = rng.standard_normal((P, f_per_part)).astype(np.float32)
    x8, xb = _prep_streams(wf, list(schedule), ACT_FRAC)
    sim = CoreSim(nc)
    sim.tensor("x8")[:] = x8
    if xb.shape[-1]:
        sim.tensor("xb")[:] = xb
    sim.simulate()
    got = float(np.array(sim.tensor("out")).astype(np.float64).sum())
    g = wf.astype(np.float64).reshape(P, f_per_part // GROUP, GROUP)
    want = float(np.sqrt((g ** 2).sum(-1)).sum())
    return abs(got - want) / abs(want)
